# revision 1
# baseline (speedup 1.0000x reference)
"""Trainium2 Bass kernel for a 2-layer Mistral-style VLM block (tensor-parallel, 8 cores).

v2 strategy (on top of the v1 LoRA/ln folding + TP sharding):
- All GEMMs in fp16 (both operands): FWL fast weight load, half the HBM
  traffic, 1 cyc/row. PSUM accumulation stays f32. Softmax normalizer is
  applied as (1/sqrt(rowsum)) twice so every scale stays in fp16 normal
  range; exp bias 0 (max exp(s) ~ 4.5e4 < 65504).
- Weights laid out [m-group, partition, k*128] in DRAM: ONE dma per slab
  (baseline: one dma per 64KB tile, 8.4k sync-queue DMAs -> ~250).
- Residual h stored fp16 in DRAM as [128, KT, S]; activations (xmega) are
  fp16 so no f32->f32r cast DMAs anywhere.
- AllReduce payloads fp16 (half the wire bytes) with Shared-addr outputs.
- Projector computed replicated on every core (kills the projector AR).
- Single-pass MLP (14 j-tiles resident in fp16), no DRAM accumulate bounce.
- rmsnorm folded into per-token scales applied to q/k/v/g/u outputs;
  scale rows broadcast to 128 partitions via PE outer-product (no DRAM
  round-trip).
- Per-batch (768-token) pipelining; AllReduce of one batch overlaps the
  other batch's compute.
"""

import os
import sys

sys.path.insert(0, '/opt/trn_rl_repo')

import numpy as np
import ml_dtypes

NCORES = 8
D, VH, DFF, NL, VOCAB, NH, NKV, HD, RK, SCALE = 4096, 1024, 14336, 2, 32000, 32, 8, 128, 8, 4.0
B, NIMG, T = 2, 257, 511
S = NIMG + T            # 768
NTOK = B * S            # 1536
DSH = D // NCORES       # 512
FSH = DFF // NCORES     # 1792
KT = D // 128           # 32
FT = FSH // 128         # 14
VK = VH // 128          # 8
QH = NH // NCORES       # 4
CH = 384
NCH = S // CH           # 2
EPS = 1e-5
ISQ = 1.0 / float(np.sqrt(HD))
EXP_BIAS = 0.0   # exp(s) <= e^10.8 < 65504 fits fp16; rowsum stays in normal range
MASK_NEG = -1e30
NIMGP = NIMG + 1          # pad to even free size

BF16 = ml_dtypes.bfloat16
F16NP = np.float16
_PROGRAM = None


def _bf(x):
    return np.ascontiguousarray(np.asarray(x, np.float32).astype(BF16))


def _h(x):
    return np.ascontiguousarray(np.asarray(x, np.float32).astype(F16NP))


def _r(x):
    """fp32 -> fp32r RNE rounding (11 explicit mantissa bits)."""
    u = np.ascontiguousarray(x, np.float32).view(np.uint32)
    low = u & np.uint32(0xFFF)
    hi = u >> np.uint32(12)
    carry = (low > 0x800) | ((low == 0x800) & ((hi & 1) == 1))
    return ((hi + carry.astype(np.uint32)) << np.uint32(12)).view(np.float32)


def _build_program():
    import concourse.bass as bass
    import concourse.bacc as bacc
    import concourse.mybir as mybir
    import concourse.tile as tile

    F32 = mybir.dt.float32
    F32R = mybir.dt.float32r
    F16 = mybir.dt.float16
    BF = mybir.dt.bfloat16
    AF = mybir.ActivationFunctionType
    ALU = mybir.AluOpType
    AF_SILU = AF.Sigmoid if os.environ.get('KSIM') == '1' else AF.Silu

    nc = bacc.Bacc("TRN2", target_bir_lowering=False)

    img_in = nc.dram_tensor("img", [128, VK * B * NIMGP], F16, kind="ExternalInput")
    projw_in = nc.dram_tensor("projw", [KT // 4, 128, 4 * VK * 128], F16, kind="ExternalInput")
    projb_in = nc.dram_tensor("projb", [128, KT], F32, kind="ExternalInput")
    txt_in = nc.dram_tensor("txt", [128, KT, B * T], F16, kind="ExternalInput")
    cos_in = nc.dram_tensor("cos_t", [128, S], F32, kind="ExternalInput")
    sin_in = nc.dram_tensor("sin_t", [128, S], F32, kind="ExternalInput")   # sign-folded
    mask_in = nc.dram_tensor("mask6", [6, 128, CH], BF, kind="ExternalInput")
    onesb_in = nc.dram_tensor("onesb", [128, 1], F16, kind="ExternalInput")
    onesrow_in = nc.dram_tensor("onesrow", [1, 128], F16, kind="ExternalInput")
    onesrow32_in = nc.dram_tensor("onesrow32", [1, 128], F32, kind="ExternalInput")
    ident_in = nc.dram_tensor("ident", [128, 128], F16, kind="ExternalInput")
    lnf_in = nc.dram_tensor("lnf", [128, KT], F32, kind="ExternalInput")
    wqkv_in = [nc.dram_tensor(f"wqkv{l}", [6, 128, KT * 128], F16, kind="ExternalInput") for l in range(NL)]
    wo_in = [nc.dram_tensor(f"wo{l}", [KT // 4, 128, 4 * QH * 128], F16, kind="ExternalInput") for l in range(NL)]
    wgu_in = [nc.dram_tensor(f"wgu{l}", [FT, 2, 128, KT * 128], F16, kind="ExternalInput") for l in range(NL)]
    wd_in = [nc.dram_tensor(f"wd{l}", [KT // 2, 128, 2 * FT * 128], F16, kind="ExternalInput") for l in range(NL)]
    out_ext = nc.dram_tensor("out", [128, KT, NTOK], F32, kind="ExternalOutput")

    RG = [list(range(NCORES))]

    with tile.TileContext(nc) as tc:
        with tc.tile_pool(name="sb", bufs=1) as sb, \
             tc.tile_pool(name="ps", bufs=1, space="PSUM") as ps, \
             tc.tile_pool(name="dram", bufs=1, space="DRAM") as dram:

            # ---- resident constants ----
            cos_sb = sb.tile([128, S], F32, tag="res_cos", bufs=1)
            sin_sb = sb.tile([128, S], F32, tag="res_sin", bufs=1)
            onesb_sb = sb.tile([128, 1], F16, tag="res_onesb", bufs=1)
            onesrow_sb = sb.tile([1, 128], F16, tag="res_onesrow", bufs=1)
            onesrow32_sb = sb.tile([1, 128], F32, tag="res_onesrow32", bufs=1)
            ident_sb = sb.tile([128, 128], F16, tag="res_ident", bufs=1)
            projb_sb = sb.tile([128, KT], F32, tag="res_projb", bufs=1)
            lnf_sb = sb.tile([128, KT], F32, tag="res_lnf", bufs=1)
            for t_, i_ in [(cos_sb, cos_in), (sin_sb, sin_in), (onesb_sb, onesb_in),
                           (onesrow_sb, onesrow_in), (onesrow32_sb, onesrow32_in),
                           (ident_sb, ident_in),
                           (projb_sb, projb_in), (lnf_sb, lnf_in)]:
                nc.sync.dma_start(t_[:], i_[:])
            mask_sb = []
            for j in range(6):
                mt_ = sb.tile([128, CH], BF, tag=f"res_mask{j}", bufs=1, name=f"msk{j}")
                nc.sync.dma_start(mt_[:], mask_in[j])
                mask_sb.append(mt_)
            eps_sb = sb.tile([128, 1], F32, tag="res_eps", bufs=1)
            nb_sb = sb.tile([128, 1], F32, tag="res_nb", bufs=1)
            nc.vector.memset(eps_sb[:], EPS)
            nc.vector.memset(nb_sb[:], EXP_BIAS)

            h_d = [dram.tile([128, KT, S], F16, tag=f"hdram{b}", bufs=1, name=f"h_d{b}")
                   for b in range(B)]

            # ---- phase 0: replicated projector (no collective); txt DMA on the
            # scalar HWDGE ring so it does not block img/weight loads on sync ----
            img_sb = sb.tile([128, VK * B * NIMGP], F16, tag="imgt", bufs=1, name="img_sb")
            nc.sync.dma_start(img_sb[:], img_in[:])
            for b in range(B):
                nc.scalar.dma_start(h_d[b][:, :, NIMG:S], txt_in[:, :, b * T:(b + 1) * T])
            for g in range(KT // 4):
                pw = sb.tile([128, 4 * VK * 128], F16, tag="wbig", bufs=3, name=f"pw{g}")
                nc.sync.dma_start(pw[:], projw_in[g])
                evs = [sb.tile([128, 4, NIMGP], F16, tag="evac", bufs=3, name=f"pje{g}{b}")
                       for b in range(B)]
                for mm in range(4):
                    m = 4 * g + mm
                    for b in range(B):
                        pt = ps.tile([128, NIMGP], F32, tag="ps1", bufs=3, name=f"pj{g}{mm}{b}")
                        for k in range(VK):
                            nc.tensor.matmul(pt[:], pw[:, (mm * VK + k) * 128:(mm * VK + k + 1) * 128],
                                             img_sb[:, (k * B + b) * NIMGP:(k * B + b + 1) * NIMGP],
                                             start=(k == 0), stop=(k == VK - 1))
                        nc.scalar.activation(evs[b][:, mm, :], pt[:], AF.Identity,
                                             bias=projb_sb[:, m:m + 1])
                for b in range(B):
                    nc.scalar.dma_start(h_d[b][:, 4 * g:4 * g + 4, 0:NIMG],
                                        evs[b][:, :, 0:NIMG])

            # =========================================================
            def bcast_pe32(row_ap, nm):
                """[1,CH] f32 row -> [128,CH] f32 SBUF via fp32 PE outer product."""
                bc = sb.tile([128, CH], F32, tag="rbc", bufs=2, name=f"b32{nm}")
                pb = ps.tile([128, CH], F32, tag="psB", bufs=1, name=f"p32{nm}")
                nc.tensor.matmul(pb[:], onesrow32_sb[:], row_ap, start=True, stop=True)
                nc.scalar.activation(bc[:], pb[:], AF.Copy)
                return bc

            def bcast_pe(row_ap, width, nm):
                """[1,width] fp16 row -> [128,width] fp16 SBUF via PE outer product."""
                bc = sb.tile([128, width], F16, tag="bc" if width == S else "rbc",
                             bufs=2, name=f"bct{nm}")
                for c0 in range(0, width, CH):
                    w = min(CH, width - c0)
                    pb = ps.tile([128, CH], F32, tag="psB", bufs=1, name=f"pb{nm}{c0}")
                    nc.tensor.matmul(pb[:, :w], onesrow_sb[:], row_ap[0:1, c0:c0 + w],
                                     start=True, stop=True)
                    nc.scalar.activation(bc[:, c0:c0 + w], pb[:, :w], AF.Copy)
                return bc

            def norm_prep(b, l, site, ar_tile, writeback=True):
                """xmega fp16 (=new h) + 1/rms broadcast f32. Updates h_d[b]."""
                xmega = sb.tile([128, KT * S], F16, tag="xmega", bufs=1, name=f"x{l}{site}{b}")
                ssq_ps = [ps.tile([1, CH], F32, tag="psS", bufs=2, name=f"sq{l}{site}{b}{c}")
                          for c in range(NCH)]
                GK = 2                       # k-tiles per slab
                for kg in range(KT // GK):
                    k0 = kg * GK
                    xsl = xmega[:, k0 * S:(k0 + GK) * S]
                    if ar_tile is not None:
                        hold = sb.tile([128, GK * S], F16, tag="tmp16", bufs=3,
                                       name=f"ho{l}{site}{b}{kg}")
                        art = sb.tile([128, GK * S], F16, tag="tmp16", bufs=3,
                                      name=f"ar{l}{site}{b}{kg}")
                        nc.sync.dma_start(hold[:], h_d[b][:, k0:k0 + GK, :])
                        nc.sync.dma_start(art[:], ar_tile[:, k0:k0 + GK, :])
                        nc.vector.tensor_tensor(xsl, hold[:], art[:], ALU.add)
                        if writeback:
                            nc.scalar.dma_start(h_d[b][:, k0:k0 + GK, :], xsl)
                    else:
                        nc.sync.dma_start(xsl, h_d[b][:, k0:k0 + GK, :])
                    sq = sb.tile([128, GK * S], F16, tag="sq16", bufs=2,
                                 name=f"s{l}{site}{b}{kg}")
                    nc.vector.tensor_tensor(sq[:], xsl, xsl, ALU.mult)
                    for kk in range(GK):
                        for c in range(NCH):
                            nc.tensor.matmul(ssq_ps[c][:], onesb_sb[:],
                                             sq[:, kk * S + c * CH:kk * S + (c + 1) * CH],
                                             start=(k0 + kk == 0), stop=(k0 + kk == KT - 1))
                s_sb = sb.tile([1, S], F32, tag="scal", bufs=2, name=f"ss{l}{site}{b}")
                r_sb = sb.tile([1, S], F16, tag="scal2", bufs=2, name=f"sr{l}{site}{b}")
                for c in range(NCH):
                    nc.scalar.activation(s_sb[:, c * CH:(c + 1) * CH], ssq_ps[c][:],
                                         AF.Sqrt, scale=1.0 / D, bias=eps_sb[0:1, :])
                with nc.allow_low_precision(reason="fp16 enough for 1/rms scales"):
                    nc.vector.reciprocal(r_sb[:], s_sb[:])
                bc = bcast_pe(r_sb[:], S, f"n{l}{site}{b}")
                return xmega, bc

            def qkv_attn(b, l, xmega, bc):
                """QKV + rope + attention -> amega (128, QH*S) fp16 resident."""
                qk_d = dram.tile([5, 128, S], F16, tag="qkd", bufs=2, name=f"qkd{l}{b}")
                vsb = sb.tile([128, S], F16, tag="vsb", bufs=2, name=f"v{l}{b}")
                for m in range(6):
                    sl = sb.tile([128, KT * 128], F16, tag="wbig", bufs=3, name=f"sq{l}{b}{m}")
                    nc.sync.dma_start(sl[:], wqkv_in[l][m])
                    if m < 5:
                        qraw = sb.tile([128, S], F16, tag="rope", bufs=5, name=f"qr{l}{b}{m}")
                    for c in range(NCH):
                        cs_ = slice(c * CH, (c + 1) * CH)
                        pt = ps.tile([128, CH], F32, tag="ps1", bufs=3, name=f"qp{l}{b}{m}{c}")
                        for k in range(KT):
                            nc.tensor.matmul(
                                pt[:], sl[:, k * 128:(k + 1) * 128],
                                xmega[:, k * S + c * CH: k * S + (c + 1) * CH],
                                start=(k == 0), stop=(k == KT - 1))
                        if m < 5:
                            nc.scalar.activation(qraw[:, cs_], pt[:], AF.Copy)
                        else:
                            nc.vector.tensor_tensor(vsb[:, cs_], pt[:], bc[:, cs_], ALU.mult)
                    if m < 5:
                        qs = sb.tile([128, S], F16, tag="rope", bufs=5, name=f"qh{l}{b}{m}")
                        nc.scalar.dma_start(qs[0:64, :], qraw[64:128, :])
                        nc.scalar.dma_start(qs[64:128, :], qraw[0:64, :])
                        t2 = sb.tile([128, S], F16, tag="rope", bufs=5, name=f"t2{l}{b}{m}")
                        nc.vector.tensor_tensor(t2[:], qraw[:], cos_sb[:], ALU.mult)
                        u2 = sb.tile([128, S], F16, tag="rope", bufs=5, name=f"u2{l}{b}{m}")
                        nc.vector.tensor_tensor(u2[:], qs[:], sin_sb[:], ALU.mult)
                        q3 = sb.tile([128, S], F16, tag="rope", bufs=5, name=f"q3{l}{b}{m}")
                        nc.vector.tensor_tensor(q3[:], t2[:], u2[:], ALU.add)
                        qf = sb.tile([128, S], F16, tag="rope", bufs=5, name=f"qf{l}{b}{m}")
                        nc.vector.tensor_tensor(qf[:], q3[:], bc[:], ALU.mult)
                        nc.scalar.dma_start(qk_d[m], qf[:])

                vtok = []
                for t in range(6):
                    trp = ps.tile([128, 128], F16, tag="ps1", bufs=3, name=f"vt{l}{b}{t}")
                    nc.tensor.transpose(trp[:], vsb[:, t * 128:(t + 1) * 128], ident_sb[:])
                    vt = sb.tile([128, 128], F16, tag="vtok", bufs=6, name=f"vk{l}{b}{t}")
                    nc.scalar.activation(vt[:], trp[:], AF.Copy)
                    vtok.append(vt)

                amega = sb.tile([128, QH * S], F16, tag="amega", bufs=1, name=f"am{l}{b}")
                ksb = sb.tile([128, S], F16, tag="qk", bufs=3, name=f"kk{l}{b}")
                nc.sync.dma_start(ksb[:], qk_d[4])
                for hh in range(QH):
                    qh_t = sb.tile([128, S], F16, tag="qk", bufs=3, name=f"ql{l}{b}{hh}")
                    nc.sync.dma_start(qh_t[:], qk_d[hh])
                    for c in range(NCH):
                        njt = 3 * (c + 1)
                        ap_ps = ps.tile([128, CH], F32, tag="psA", bufs=2, name=f"ap{l}{b}{hh}{c}")
                        ss_ps = ps.tile([1, CH], F32, tag="psS", bufs=2, name=f"sm{l}{b}{hh}{c}")
                        for jt in range(njt):
                            sc = ps.tile([128, CH], F32, tag="ps1", bufs=3, name=f"sc{l}{b}{hh}{c}{jt}")
                            nc.tensor.matmul(sc[:], ksb[:, jt * 128:(jt + 1) * 128],
                                             qh_t[:, c * CH:(c + 1) * CH],
                                             start=True, stop=True)
                            et = sb.tile([128, CH], F16, tag="expT", bufs=3, name=f"et{l}{b}{hh}{c}{jt}")
                            if jt >= 3 * c:
                                madd = sb.tile([128, CH], F32, tag="madd", bufs=2, name=f"md{l}{b}{hh}{c}{jt}")
                                nc.vector.tensor_tensor(madd[:], sc[:], mask_sb[jt][:], ALU.add)
                                nc.scalar.activation(et[:], madd[:], AF.Exp, scale=ISQ, bias=nb_sb[:])
                            else:
                                nc.scalar.activation(et[:], sc[:], AF.Exp, scale=ISQ, bias=nb_sb[:])
                            nc.tensor.matmul(ss_ps[:], onesb_sb[:], et[:],
                                             start=(jt == 0), stop=(jt == njt - 1))
                            nc.tensor.matmul(ap_ps[:], vtok[jt][:], et[:],
                                             start=(jt == 0), stop=(jt == njt - 1))
                        rec = sb.tile([1, CH], F32, tag="scal", bufs=2, name=f"rc{l}{b}{hh}{c}")
                        nc.vector.reciprocal(rec[:], ss_ps[:])
                        rbc = bcast_pe32(rec[:], f"a{l}{b}{hh}{c}")
                        nc.vector.tensor_tensor(
                            amega[:, hh * S + c * CH: hh * S + (c + 1) * CH],
                            ap_ps[:], rbc[:], ALU.mult)
                return amega

            def wo_site(b, l, amega):
                """Wo row-parallel partial + fp16 AllReduce."""
                arin = dram.tile([128, KT, S], F16, tag="arin", bufs=2, name=f"ai{l}o{b}")
                arout = dram.tile([128, KT, S], F16, tag="arout", bufs=2,
                                  addr_space="Shared", name=f"ao{l}o{b}")
                for g in range(KT // 4):
                    sl = sb.tile([128, 4 * QH * 128], F16, tag="wbig", bufs=3, name=f"so{l}{b}{g}")
                    nc.sync.dma_start(sl[:], wo_in[l][g])
                    for half in range(2):
                        ev = sb.tile([128, 2, S], F16, tag="evac", bufs=3, name=f"oe{l}{b}{g}{half}")
                        for mm2 in range(2):
                            mm = half * 2 + mm2
                            for c in range(NCH):
                                pt = ps.tile([128, CH], F32, tag="ps1", bufs=3,
                                             name=f"o{l}{b}{g}{mm}{c}")
                                for k in range(QH):
                                    nc.tensor.matmul(
                                        pt[:], sl[:, (mm * QH + k) * 128:(mm * QH + k + 1) * 128],
                                        amega[:, k * S + c * CH: k * S + (c + 1) * CH],
                                        start=(k == 0), stop=(k == QH - 1))
                                nc.scalar.activation(ev[:, mm2, c * CH:(c + 1) * CH], pt[:], AF.Copy)
                        nc.scalar.dma_start(arin[:, 4 * g + 2 * half:4 * g + 2 * half + 2, :], ev[:])
                nc.gpsimd.collective_compute("AllReduce", ALU.add, replica_groups=RG,
                                             ins=[arin.opt()], outs=[arout.opt()])
                return arout

            def mlp_site(b, l, xmega, bc):
                """gate/up gemms + silu*u + single-pass down-proj + fp16 AllReduce."""
                arin = dram.tile([128, KT, S], F16, tag="arin", bufs=2, name=f"ai{l}d{b}")
                arout = dram.tile([128, KT, S], F16, tag="arout", bufs=2,
                                  addr_space="Shared", name=f"ao{l}d{b}")
                mts = {}
                for j in range(FT):
                    gt = sb.tile([128, S], F16, tag="tmp16", bufs=3, name=f"gs{l}{b}{j}")
                    ut = sb.tile([128, S], F16, tag="tmp16", bufs=3, name=f"us{l}{b}{j}")
                    for gu in range(2):
                        sl = sb.tile([128, KT * 128], F16, tag="wbig", bufs=3,
                                     name=f"sg{l}{b}{j}{gu}")
                        nc.sync.dma_start(sl[:], wgu_in[l][j, gu])
                        dst = gt if gu == 0 else ut
                        for c in range(NCH):
                            cs_ = slice(c * CH, (c + 1) * CH)
                            pt = ps.tile([128, CH], F32, tag="ps1", bufs=3,
                                         name=f"g{l}{b}{j}{gu}{c}")
                            for k in range(KT):
                                nc.tensor.matmul(
                                    pt[:], sl[:, k * 128:(k + 1) * 128],
                                    xmega[:, k * S + c * CH: k * S + (c + 1) * CH],
                                    start=(k == 0), stop=(k == KT - 1))
                            nc.vector.tensor_tensor(dst[:, cs_], pt[:], bc[:, cs_], ALU.mult)
                    sil = sb.tile([128, S], F16, tag="tmp16", bufs=3, name=f"si{l}{b}{j}")
                    nc.scalar.activation(sil[:], gt[:], AF_SILU)
                    mt = sb.tile([128, S], F16, tag="mstream", bufs=FT + 1, name=f"mt{l}{b}{j}")
                    nc.vector.tensor_tensor(mt[:], sil[:], ut[:], ALU.mult)
                    mts[j] = mt
                for g in range(KT // 2):
                    sl = sb.tile([128, 2 * FT * 128], F16, tag="wbig", bufs=3,
                                 name=f"sd{l}{b}{g}")
                    nc.sync.dma_start(sl[:], wd_in[l][g])
                    ev = sb.tile([128, 2, S], F16, tag="evac", bufs=3, name=f"de{l}{b}{g}")
                    for mm in range(2):
                        for c in range(NCH):
                            pt = ps.tile([128, CH], F32, tag="ps1", bufs=3,
                                         name=f"dp{l}{b}{g}{mm}{c}")
                            for k in range(FT):
                                nc.tensor.matmul(
                                    pt[:], sl[:, (mm * FT + k) * 128:(mm * FT + k + 1) * 128],
                                    mts[k][:, c * CH:(c + 1) * CH],
                                    start=(k == 0), stop=(k == FT - 1))
                            nc.scalar.activation(ev[:, mm, c * CH:(c + 1) * CH], pt[:], AF.Copy)
                    nc.scalar.dma_start(arin[:, 2 * g:2 * g + 2, :], ev[:])
                nc.gpsimd.collective_compute("AllReduce", ALU.add, replica_groups=RG,
                                             ins=[arin.opt()], outs=[arout.opt()])
                return arout

            def final_norm(b, ar_tile):
                xmega, bc = norm_prep(b, 9, 'f', ar_tile, writeback=False)
                for kg in range(KT // 2):
                    ot = sb.tile([128, 2, S], F32, tag="otile", bufs=2, name=f"ot{b}{kg}")
                    for kk in range(2):
                        k = kg * 2 + kk
                        nc.vector.scalar_tensor_tensor(
                            ot[:, kk, :], xmega[:, k * S:(k + 1) * S],
                            lnf_sb[:, k:k + 1], bc[:], ALU.mult, ALU.mult)
                    nc.sync.dma_start(out_ext[:, kg * 2:kg * 2 + 2, b * S:(b + 1) * S], ot[:])

            # ---- main schedule ----
            ar_pending = [None, None]
            for l in range(NL):
                ar_o = [None, None]
                for b in range(B):
                    xmega, bc = norm_prep(b, l, 'a', ar_pending[b])
                    amega = qkv_attn(b, l, xmega, bc)
                    ar_o[b] = wo_site(b, l, amega)
                for b in range(B):
                    xmega, bc = norm_prep(b, l, 'm', ar_o[b])
                    ar_pending[b] = mlp_site(b, l, xmega, bc)
            for b in range(B):
                final_norm(b, ar_pending[b])

    nc.compile()
    return nc


def _host_prep(inputs):
    I = {k: np.asarray(v) for k, v in inputs.items()}

    def fold(W, A, Bm, lnw=None):
        W64 = W.astype(np.float64) + SCALE * (Bm.astype(np.float64) @ A.astype(np.float64))
        if lnw is not None:
            W64 = W64 * lnw.astype(np.float64)[None, :]
        return W64.astype(np.float32)

    ids = np.asarray(I['input_ids'], np.int64)
    txt = I['embed'][ids]                                    # (B, T, D)
    txtT = txt.reshape(B * T, D).T.astype(np.float32)        # (D, B*T) order: b-major cols
    # cols must be [b*T + t]; txt.reshape gives rows (b,t) -> .T cols (b*T+t)  OK
    txt16 = _h(txtT).reshape(KT, 128, B * T).transpose(1, 0, 2)   # [128, KT, B*T]

    inv = 1.0 / (10000.0 ** (np.arange(0, HD, 2, dtype=np.float64) / HD))
    ang = np.arange(S, dtype=np.float64)[:, None] * inv[None, :]
    cosT = np.ascontiguousarray(np.concatenate([np.cos(ang), np.cos(ang)], 1).T).astype(np.float32)
    sinT = np.ascontiguousarray(np.concatenate([-np.sin(ang), np.sin(ang)], 1).T).astype(np.float32)

    mask6 = np.zeros((6, 128, CH), np.float32)
    for jt in range(6):
        c = 0 if jt < 3 else 1
        jj = np.arange(jt * 128, (jt + 1) * 128)[:, None]
        ii = np.arange(c * CH, (c + 1) * CH)[None, :]
        mask6[jt] = np.where(jj <= ii, 0.0, MASK_NEG)

    imgT = I['image_embeds'].reshape(B * NIMG, VH).T.astype(np.float32)   # (VH, B*NIMG)
    imp = np.zeros((VK, 128, B * NIMGP), np.float32)
    for k in range(VK):
        for b in range(B):
            imp[k, :, b * NIMGP:b * NIMGP + NIMG] = imgT[k * 128:(k + 1) * 128,
                                                         b * NIMG:(b + 1) * NIMG]
    img16 = _h(imp.transpose(1, 0, 2).reshape(128, VK * B * NIMGP))

    projT = I['proj_W'].astype(np.float32).T                 # (VH, D)
    # [g, p, (mm*VK+k)*128+f]
    pw = projT.reshape(VK, 128, KT, 128).transpose(2, 1, 0, 3)        # [m,p,k,f]
    pw = pw.reshape(KT // 4, 4, 128, VK, 128).transpose(0, 2, 1, 3, 4)
    projw16 = _h(pw.reshape(KT // 4, 128, 4 * VK * 128))

    projb_t = np.ascontiguousarray(I['proj_b'].astype(np.float32).reshape(KT, 128).T)
    lnf_t = np.ascontiguousarray(I['ln_f'].astype(np.float32).reshape(KT, 128).T)

    shared = dict(
        projb=projb_t, txt=np.ascontiguousarray(txt16), cos_t=cosT, sin_t=sinT,
        mask6=_bf(mask6),
        onesb=_h(np.ones((128, 1), np.float32)),
        onesrow=_h(np.ones((1, 128), np.float32)),
        onesrow32=np.ones((1, 128), np.float32),
        ident=_h(np.eye(128, dtype=np.float32)),
        lnf=lnf_t,
        img=img16, projw=projw16,
    )

    per_core = [dict(shared) for _ in range(NCORES)]

    for l in range(NL):
        Wq = fold(I['Wq'][l], I['Aq'][l], I['Bq'][l], I['ln1'][l])
        Wk = fold(I['Wk'][l], I['Ak'][l], I['Bk'][l], I['ln1'][l])
        Wv = fold(I['Wv'][l], I['Av'][l], I['Bv'][l], I['ln1'][l])
        Wo = fold(I['Wo'][l], I['Ao'][l], I['Bo'][l])
        Wg = fold(I['Wg'][l], I['Ag'][l], I['Bg'][l], I['ln2'][l])
        Wu = fold(I['Wu'][l], I['Au'][l], I['Bu'][l], I['ln2'][l])
        Wd = fold(I['Wd'][l], I['Ad'][l], I['Bd'][l])
        for r in range(NCORES):
            qs = Wq[r * DSH:(r + 1) * DSH]
            ks = Wk[r * HD:(r + 1) * HD]
            vs = Wv[r * HD:(r + 1) * HD]
            wqkvT = np.vstack([qs, ks, vs]).T                # (D, 768)
            arr = wqkvT.reshape(KT, 128, 6, 128).transpose(2, 1, 0, 3)   # [m,p,k,f]
            per_core[r][f"wqkv{l}"] = _h(arr.reshape(6, 128, KT * 128))

            woT = Wo[:, r * DSH:(r + 1) * DSH].T             # (512, D) rows=k-in, cols=d-out
            arr = woT.reshape(QH, 128, KT, 128).transpose(2, 1, 0, 3)    # [m,p,k,f]
            arr = arr.reshape(KT // 4, 4, 128, QH, 128).transpose(0, 2, 1, 3, 4)
            per_core[r][f"wo{l}"] = _h(arr.reshape(KT // 4, 128, 4 * QH * 128))

            gT = Wg[r * FSH:(r + 1) * FSH].T                 # (D, FSH)
            uT = Wu[r * FSH:(r + 1) * FSH].T
            ga = gT.reshape(KT, 128, FT, 128).transpose(2, 1, 0, 3).reshape(FT, 128, KT * 128)
            ua = uT.reshape(KT, 128, FT, 128).transpose(2, 1, 0, 3).reshape(FT, 128, KT * 128)
            per_core[r][f"wgu{l}"] = _h(np.stack([ga, ua], axis=1))

            wdT = Wd[:, r * FSH:(r + 1) * FSH].T             # (FSH, D)
            arr = wdT.reshape(FT, 128, KT, 128).transpose(2, 1, 0, 3)    # [m,p,j,f]
            arr = arr.reshape(KT // 2, 2, 128, FT, 128).transpose(0, 2, 1, 3, 4)
            per_core[r][f"wd{l}"] = _h(arr.reshape(KT // 2, 128, 2 * FT * 128))
    return per_core


def kernel(**inputs):
    global _PROGRAM
    from concourse.bass_utils import run_bass_kernel_spmd

    in_maps = _host_prep(inputs)
    if _PROGRAM is None:
        _PROGRAM = _build_program()
    res = None
    for attempt in range(3):
        try:
            res = run_bass_kernel_spmd(_PROGRAM, in_maps, list(range(NCORES)))
            break
        except Exception as e:
            if attempt == 2 or 'UNAVAILABLE' not in str(type(e).__name__) + str(e):
                raise
    out = np.asarray(res.results[0]["out"], np.float32)      # [128, KT, B*S]
    full = out.reshape(128, KT, B, S).transpose(2, 3, 1, 0)  # (B, S, KT, 128)
    return np.ascontiguousarray(full.reshape(B, S, D))

